# revision 10
# baseline (speedup 1.0000x reference)
"""Trainium2 Bass kernel for nn_BeamSearchDecoder_46093589021017.

Structure of the problem: the reference beam search over lprobs [1, T=4096,
V=10000] only ever consumes
  - topi[t] = indices of the top-5 lprobs of row t     (the heavy part), and
  - raw5[t] = lprobs[t, :5]                            (a pure slice),
after which a tiny sequential 3-beam scan (T-1 steps over [3,5] candidates)
produces the output. The memory-bound part (reading 164 MB once) runs on the
8 NeuronCores, sharded over the T axis (512 rows/core).

Device algorithm per row (exact, one DVE pass over the data instead of the
two full passes max8+find_index8 would need):
  1. segmented max-reduce: 400 segment maxes of width 25 (1 elem/cycle DVE)
  2. max8 + find_index8 over the 400 segment maxes (tiny)
  -> top-8 (segment max value, segment id) pairs per row.
Host: the true top-5 values of a row live in segments whose max is >= the
5th-largest segment max (s5 <= v5 always), i.e. within the top-8 segments
returned. Gather those 8x25 columns from the input (host already holds it)
and take the exact top-5. Rows where the 8 segment maxes are not strictly
distinct (could hide a tie at s5) fall back to an exact host re-scan of the
row; with continuous data this never triggers.

The [T,5] -> ([3],[3,T]) beam scan is inherently sequential tiny bookkeeping
and runs on host in exact float32, replicating the reference op-for-op
(including JAX's out-of-bounds index clamping).
"""

import numpy as np

import concourse.bass as bass
import concourse.mybir as mybir
from concourse.bass_utils import run_bass_kernel_spmd

# Problem shapes (hardcoded per contract; kernel.py must be self-contained).
T = 4096
V = 10000
N_CORES = 8
ROWS = T // N_CORES  # rows (timesteps) per core
P = 128  # SBUF partitions
N_TILES = ROWS // P  # row-tiles per core

SW = 25  # segment width for the first-level max-reduce
SEGS = V // SW  # 400 segments per row
W = 2500  # columns per steady-state load chunk
NSLOTS = 8  # load buffering depth
# per-tile chunk column widths; tile 0 leads with small chunks so the DVE
# can start reducing ~3us earlier (queue ring spin-up dominates the ramp)
_TILE0_CHUNKS = [625, 625, 625, 625, 2500, 2500, 2500]
_TILE_CHUNKS = [2500, 2500, 2500, 2500]


def _chunk_plan():
    """[(tile, col_start, width, is_last_chunk_of_tile)] in issue order."""
    plan = []
    for t in range(N_TILES):
        widths = _TILE0_CHUNKS if t == 0 else _TILE_CHUNKS
        assert sum(widths) == V
        col = 0
        for k, w in enumerate(widths):
            plan.append((t, col, w, k == len(widths) - 1))
            col += w
    return plan

BEAM_WIDTH = 5
NSENT = 3
SOS = 1
EOS = 2

_BASS_CACHE = {}


def _build_topk_bass():
    """One-core Bass program: per-row top-8 (segment max, segment id) of
    lp [ROWS, V] with segments of width SW.

    Hand-rolled pipeline (no TileContext: its final drain would need 9 sem
    waits — DVE + 8 DMA-HW queues — exceeding the codegen per-instruction
    wait limit). Loads on SP, reduce/max8/find-index8 on DVE, stores on
    Activation.
    """
    nc = bass.Bass("TRN2")
    lp = nc.declare_dram_parameter("lp", [ROWS, V], mybir.dt.float32, isOutput=False)
    outv = nc.declare_dram_parameter("segv", [ROWS, 8], mybir.dt.float32, isOutput=True)
    outi = nc.declare_dram_parameter("segi", [ROWS, 8], mybir.dt.uint32, isOutput=True)
    bufs = [
        nc.alloc_sbuf_tensor(f"buf{s}", [P, W], mybir.dt.float32)
        for s in range(NSLOTS)
    ]
    segmax = [
        nc.alloc_sbuf_tensor(f"segmax{t}", [P, SEGS], mybir.dt.float32)
        for t in range(N_TILES)
    ]
    segv = [
        nc.alloc_sbuf_tensor(f"segv{t}", [P, 8], mybir.dt.float32)
        for t in range(N_TILES)
    ]
    segi = [
        nc.alloc_sbuf_tensor(f"segi{t}", [P, 8], mybir.dt.uint32)
        for t in range(N_TILES)
    ]
    plan = _chunk_plan()
    NCHUNKS = len(plan)
    with (
        nc.Block() as block,
        nc.semaphore("load_sem0") as load_sem0,
        nc.semaphore("load_sem1") as load_sem1,
        nc.semaphore("load_sem2") as load_sem2,
        nc.semaphore("load_sem3") as load_sem3,
        nc.semaphore("load_sem4") as load_sem4,
        nc.semaphore("load_sem5") as load_sem5,
        nc.semaphore("load_sem6") as load_sem6,
        nc.semaphore("load_sem7") as load_sem7,
        nc.semaphore("red_sem") as red_sem,
        nc.semaphore("max_sem") as max_sem,
        nc.semaphore("vec_sem") as vec_sem,
        nc.semaphore("store_sem") as store_sem,
    ):
        # one load semaphore per buffer slot: descriptor completions of
        # different in-flight DMAs interleave on a shared semaphore, so a
        # shared counter cannot tell which DMA finished
        load_sems = [load_sem0, load_sem1, load_sem2, load_sem3, load_sem4, load_sem5, load_sem6, load_sem7]

        @block.sync
        def _(sync):
            for ci, (t, col, w, _last) in enumerate(plan):
                if ci >= NSLOTS:
                    # slot ci%NSLOTS is free once DVE reduced chunk ci-NSLOTS
                    sync.wait_ge(red_sem, ci - NSLOTS + 1)
                sync.dma_start(
                    out=bufs[ci % NSLOTS][:, :w],
                    in_=lp[t * P : (t + 1) * P, col : col + w],
                ).then_inc(load_sems[ci % NSLOTS], 16)

        @block.vector
        def _(vector):
            nfin = 0
            for ci, (t, col, w, last) in enumerate(plan):
                s = ci % NSLOTS
                vector.wait_ge(load_sems[s], 16 * (ci // NSLOTS + 1))
                vector.reduce_max(
                    out=segmax[t][:, col // SW : (col + w) // SW],
                    in_=bufs[s][:, :w].rearrange("p (g w) -> p g w", w=SW),
                    axis=mybir.AxisListType.X,
                ).then_inc(red_sem, 1)
                if last:
                    nfin += 1
                    vector.wait_ge(red_sem, ci + 1)
                    vector.max(out=segv[t][:], in_=segmax[t][:]).then_inc(max_sem, 1)
                    vector.wait_ge(max_sem, nfin)
                    vector.max_index(
                        out=segi[t][:], in_max=segv[t][:], in_values=segmax[t][:]
                    ).then_inc(vec_sem, 1)

        @block.scalar
        def _(act):
            for t in range(N_TILES):
                act.wait_ge(vec_sem, t + 1)
                act.dma_start(out=outv[t * P : (t + 1) * P, :], in_=segv[t][:]).then_inc(
                    store_sem, 16
                )
                act.dma_start(out=outi[t * P : (t + 1) * P, :], in_=segi[t][:]).then_inc(
                    store_sem, 16
                )
            act.wait_ge(store_sem, 16 * 2 * N_TILES)

    return nc


def _device_seg8(lp_full: np.ndarray, trace: bool = False):
    """lp_full: [T, V] f32 -> (segv [T,8] f32, segi [T,8] uint32, exec_ns)."""
    if "nc" not in _BASS_CACHE:
        _BASS_CACHE["nc"] = _build_topk_bass()
    nc = _BASS_CACHE["nc"]
    in_maps = [
        {"lp": np.ascontiguousarray(lp_full[c * ROWS : (c + 1) * ROWS])}
        for c in range(N_CORES)
    ]
    out = run_bass_kernel_spmd(nc, in_maps, core_ids=list(range(N_CORES)), trace=trace)
    segv = np.concatenate([out.results[c]["segv"] for c in range(N_CORES)], axis=0)
    segi = np.concatenate([out.results[c]["segi"] for c in range(N_CORES)], axis=0)
    return segv, segi, out.exec_time_ns


def _host_top5_from_segments(lp_full, segv, segi):
    """Exact per-row top-5 indices from the device's top-8 segment hits."""
    n = lp_full.shape[0]
    # guard: duplicated segment-max values (descending order => adjacent) or
    # out-of-range segment ids would void the coverage proof -> exact fallback
    bad = (segv[:, 1:] == segv[:, :-1]).any(axis=1) | (
        segi.astype(np.int64) >= SEGS
    ).any(axis=1)
    segs_sorted = np.sort(segi.astype(np.int64), axis=1)  # ascending segment ids
    cols = (
        segs_sorted[:, :, None] * SW + np.arange(SW, dtype=np.int64)[None, None, :]
    ).reshape(n, 8 * SW)
    vals = np.take_along_axis(lp_full, cols, axis=1)
    ord5 = np.argsort(-vals, axis=1, kind="stable")[:, :BEAM_WIDTH]
    topi = np.take_along_axis(cols, ord5, axis=1).astype(np.int32)
    if bad.any():
        for r in np.flatnonzero(bad):
            topi[r] = np.argsort(-lp_full[r], kind="stable")[:BEAM_WIDTH]
    return topi


def _host_beam_scan(raw5: np.ndarray, topi: np.ndarray, seq_len_T: int):
    """Exact float32 port of the reference _beam_search scan, consuming
    precomputed raw5 [T,5] (lprobs[:, :5]) and topi [T,5] (top-5 indices)."""
    dt = np.float32
    BIGF = np.float32(1e30)
    F1 = np.float32(1.0)
    EPS = np.float32(1e-6)
    HALF_BIG = np.float32(1e30 * 0.5)
    OFFS = np.float32(1e15)

    lp = np.zeros(NSENT, dt)
    lp[0] = raw5[0, 0]
    ln = np.array([1, 0, 0], np.int32)
    tok = np.array([SOS, 0, 0], np.int32)
    alive = np.array([True, False, False])
    seqs = np.zeros((NSENT, seq_len_T), np.int32)
    seqs[0, 0] = SOS
    fsc = np.full(NSENT, BIGF, dt)
    fseq = np.zeros((NSENT, seq_len_T), np.int32)

    for t in range(1, seq_len_T):
        raw = raw5[t]
        toki = topi[t]
        is_eos = alive & (tok == EOS) & (ln > 1)
        if is_eos.any():
            ev = -(lp / (ln.astype(dt) - F1 + EPS))
            cand_f = np.where(is_eos, ev, BIGF)
            all_sc = np.concatenate([fsc, cand_f])
            all_seq = np.concatenate([fseq, seqs], axis=0)
            keep = np.argsort(all_sc, kind="stable")[:NSENT]
            fsc, fseq = all_sc[keep], all_seq[keep]
        expand = alive & ~is_eos
        cl = lp[:, None] + raw[None, :]
        clen = ln[:, None] + 1  # shape [3, 1], as in the reference
        csc = -(cl / (clen.astype(dt) - F1 + EPS))
        csc = np.where(expand[:, None], csc, BIGF).reshape(-1)
        sel = np.argsort(csc, kind="stable")[:NSENT]
        parent = sel // BEAM_WIDTH
        lp = cl.reshape(-1)[sel]
        # reference does clen.reshape(-1)[sel] on a 3-element array under
        # JAX out-of-bounds clamping semantics
        ln = clen.reshape(-1)[np.minimum(sel, NSENT - 1)]
        tok = toki[sel % BEAM_WIDTH]
        alive = csc[sel] < HALF_BIG
        seqs = seqs[parent].copy()
        seqs[:, t] = tok

    beam_sc = np.where(alive, -(lp / (ln.astype(dt) - F1 + EPS)), BIGF)
    rank_sc = np.concatenate([fsc, beam_sc + OFFS])
    true_sc = np.concatenate([fsc, beam_sc])
    all_seq = np.concatenate([fseq, seqs], axis=0)
    order = np.argsort(rank_sc, kind="stable")[:NSENT]
    return true_sc[order], all_seq[order]


def kernel(predicted_seq_lprobs, target_seq, seq_len):
    lp_full = np.ascontiguousarray(
        np.asarray(predicted_seq_lprobs, dtype=np.float32)[0]
    )  # [T, V]
    segv, segi, _ = _device_seg8(lp_full)
    topi = _host_top5_from_segments(lp_full, segv, segi)
    raw5 = lp_full[:, :BEAM_WIDTH]
    scores, seqs = _host_beam_scan(raw5, topi, T)
    return seqs.astype(np.int32), scores.astype(np.float32)


# revision 11
# speedup vs baseline: 1.1636x; 1.1636x over previous
"""Trainium2 Bass kernel for nn_BeamSearchDecoder_46093589021017.

Structure of the problem: the reference beam search over lprobs [1, T=4096,
V=10000] only ever consumes
  - topi[t] = indices of the top-5 lprobs of row t     (the heavy part), and
  - raw5[t] = lprobs[t, :5]                            (a pure slice),
after which a tiny sequential 3-beam scan (T-1 steps over [3,5] candidates)
produces the output. The memory-bound part (reading 164 MB once) runs on the
8 NeuronCores, sharded over the T axis (512 rows/core).

Device algorithm per row (exact, one DVE pass over the data instead of the
two full passes max8+find_index8 would need):
  1. segmented max-reduce: 400 segment maxes of width 25 (1 elem/cycle DVE)
  2. max8 + find_index8 over the 400 segment maxes (tiny)
  -> top-8 (segment max value, segment id) pairs per row.
Host: the true top-5 values of a row live in segments whose max is >= the
5th-largest segment max (s5 <= v5 always), i.e. within the top-8 segments
returned. Gather those 8x25 columns from the input (host already holds it)
and take the exact top-5. Rows where the 8 segment maxes are not strictly
distinct (could hide a tie at s5) fall back to an exact host re-scan of the
row; with continuous data this never triggers.

The [T,5] -> ([3],[3,T]) beam scan is inherently sequential tiny bookkeeping
and runs on host in exact float32, replicating the reference op-for-op
(including JAX's out-of-bounds index clamping).
"""

import numpy as np

import concourse.bass as bass
import concourse.mybir as mybir
from concourse.bass_utils import run_bass_kernel_spmd

# Problem shapes (hardcoded per contract; kernel.py must be self-contained).
T = 4096
V = 10000
N_CORES = 8
ROWS = T // N_CORES  # rows (timesteps) per core
P = 128  # SBUF partitions
N_TILES = ROWS // P  # row-tiles per core

SW = 25  # segment width for the first-level max-reduce
SEGS = V // SW  # 400 segments per row
W = 2500  # columns per steady-state load chunk
NSLOTS = 8  # load buffering depth
# per-tile chunk column widths; tile 0 leads with small chunks so the DVE
# can start reducing ~3us earlier (queue ring spin-up dominates the ramp)
_TILE0_CHUNKS = [2500, 2500, 2500, 2500]
_TILE_CHUNKS = [2500, 2500, 2500, 2500]


def _chunk_plan():
    """[(tile, col_start, width, is_last_chunk_of_tile)] in issue order."""
    plan = []
    for t in range(N_TILES):
        widths = _TILE0_CHUNKS if t == 0 else _TILE_CHUNKS
        assert sum(widths) == V
        col = 0
        for k, w in enumerate(widths):
            plan.append((t, col, w, k == len(widths) - 1))
            col += w
    return plan

BEAM_WIDTH = 5
NSENT = 3
SOS = 1
EOS = 2

_BASS_CACHE = {}


def _build_topk_bass():
    """One-core Bass program: per-row top-8 (segment max, segment id) of
    lp [ROWS, V] with segments of width SW.

    Hand-rolled pipeline (no TileContext: its final drain would need 9 sem
    waits — DVE + 8 DMA-HW queues — exceeding the codegen per-instruction
    wait limit). Loads on SP, reduce/max8/find-index8 on DVE, stores on
    Activation.
    """
    nc = bass.Bass("TRN2")
    lp = nc.declare_dram_parameter("lp", [ROWS, V], mybir.dt.float32, isOutput=False)
    outv = nc.declare_dram_parameter("segv", [ROWS, 8], mybir.dt.float32, isOutput=True)
    outi = nc.declare_dram_parameter("segi", [ROWS, 8], mybir.dt.uint32, isOutput=True)
    bufs = [
        nc.alloc_sbuf_tensor(f"buf{s}", [P, W], mybir.dt.float32)
        for s in range(NSLOTS)
    ]
    segmax = [
        nc.alloc_sbuf_tensor(f"segmax{t}", [P, SEGS], mybir.dt.float32)
        for t in range(N_TILES)
    ]
    segv = [
        nc.alloc_sbuf_tensor(f"segv{t}", [P, 8], mybir.dt.float32)
        for t in range(N_TILES)
    ]
    segi = [
        nc.alloc_sbuf_tensor(f"segi{t}", [P, 8], mybir.dt.uint32)
        for t in range(N_TILES)
    ]
    plan = _chunk_plan()
    NCHUNKS = len(plan)
    with (
        nc.Block() as block,
        nc.semaphore("load_sem0") as load_sem0,
        nc.semaphore("load_sem1") as load_sem1,
        nc.semaphore("load_sem2") as load_sem2,
        nc.semaphore("load_sem3") as load_sem3,
        nc.semaphore("load_sem4") as load_sem4,
        nc.semaphore("load_sem5") as load_sem5,
        nc.semaphore("load_sem6") as load_sem6,
        nc.semaphore("load_sem7") as load_sem7,
        nc.semaphore("red_sem") as red_sem,
        nc.semaphore("max_sem") as max_sem,
        nc.semaphore("vec_sem") as vec_sem,
        nc.semaphore("store_sem") as store_sem,
    ):
        # one load semaphore per buffer slot: descriptor completions of
        # different in-flight DMAs interleave on a shared semaphore, so a
        # shared counter cannot tell which DMA finished
        load_sems = [load_sem0, load_sem1, load_sem2, load_sem3, load_sem4, load_sem5, load_sem6, load_sem7]

        @block.sync
        def _(sync):
            for ci, (t, col, w, _last) in enumerate(plan):
                if ci >= NSLOTS:
                    # slot ci%NSLOTS is free once DVE reduced chunk ci-NSLOTS
                    sync.wait_ge(red_sem, ci - NSLOTS + 1)
                sync.dma_start(
                    out=bufs[ci % NSLOTS][:, :w],
                    in_=lp[t * P : (t + 1) * P, col : col + w],
                ).then_inc(load_sems[ci % NSLOTS], 16)

        @block.vector
        def _(vector):
            nfin = 0
            for ci, (t, col, w, last) in enumerate(plan):
                s = ci % NSLOTS
                vector.wait_ge(load_sems[s], 16 * (ci // NSLOTS + 1))
                vector.reduce_max(
                    out=segmax[t][:, col // SW : (col + w) // SW],
                    in_=bufs[s][:, :w].rearrange("p (g w) -> p g w", w=SW),
                    axis=mybir.AxisListType.X,
                ).then_inc(red_sem, 1)
                if last:
                    nfin += 1
                    vector.wait_ge(red_sem, ci + 1)
                    vector.max(out=segv[t][:], in_=segmax[t][:]).then_inc(max_sem, 1)
                    vector.wait_ge(max_sem, nfin)
                    vector.max_index(
                        out=segi[t][:], in_max=segv[t][:], in_values=segmax[t][:]
                    ).then_inc(vec_sem, 1)

        @block.scalar
        def _(act):
            for t in range(N_TILES):
                act.wait_ge(vec_sem, t + 1)
                act.dma_start(out=outv[t * P : (t + 1) * P, :], in_=segv[t][:]).then_inc(
                    store_sem, 16
                )
                act.dma_start(out=outi[t * P : (t + 1) * P, :], in_=segi[t][:]).then_inc(
                    store_sem, 16
                )
            act.wait_ge(store_sem, 16 * 2 * N_TILES)

    return nc


def _device_seg8(lp_full: np.ndarray, trace: bool = False):
    """lp_full: [T, V] f32 -> (segv [T,8] f32, segi [T,8] uint32, exec_ns)."""
    if "nc" not in _BASS_CACHE:
        _BASS_CACHE["nc"] = _build_topk_bass()
    nc = _BASS_CACHE["nc"]
    in_maps = [
        {"lp": np.ascontiguousarray(lp_full[c * ROWS : (c + 1) * ROWS])}
        for c in range(N_CORES)
    ]
    out = run_bass_kernel_spmd(nc, in_maps, core_ids=list(range(N_CORES)), trace=trace)
    segv = np.concatenate([out.results[c]["segv"] for c in range(N_CORES)], axis=0)
    segi = np.concatenate([out.results[c]["segi"] for c in range(N_CORES)], axis=0)
    return segv, segi, out.exec_time_ns


def _host_top5_from_segments(lp_full, segv, segi):
    """Exact per-row top-5 indices from the device's top-8 segment hits."""
    n = lp_full.shape[0]
    # guard: duplicated segment-max values (descending order => adjacent) or
    # out-of-range segment ids would void the coverage proof -> exact fallback
    bad = (segv[:, 1:] == segv[:, :-1]).any(axis=1) | (
        segi.astype(np.int64) >= SEGS
    ).any(axis=1)
    segs_sorted = np.sort(segi.astype(np.int64), axis=1)  # ascending segment ids
    cols = (
        segs_sorted[:, :, None] * SW + np.arange(SW, dtype=np.int64)[None, None, :]
    ).reshape(n, 8 * SW)
    vals = np.take_along_axis(lp_full, cols, axis=1)
    ord5 = np.argsort(-vals, axis=1, kind="stable")[:, :BEAM_WIDTH]
    topi = np.take_along_axis(cols, ord5, axis=1).astype(np.int32)
    if bad.any():
        for r in np.flatnonzero(bad):
            topi[r] = np.argsort(-lp_full[r], kind="stable")[:BEAM_WIDTH]
    return topi


def _host_beam_scan(raw5: np.ndarray, topi: np.ndarray, seq_len_T: int):
    """Exact float32 port of the reference _beam_search scan, consuming
    precomputed raw5 [T,5] (lprobs[:, :5]) and topi [T,5] (top-5 indices)."""
    dt = np.float32
    BIGF = np.float32(1e30)
    F1 = np.float32(1.0)
    EPS = np.float32(1e-6)
    HALF_BIG = np.float32(1e30 * 0.5)
    OFFS = np.float32(1e15)

    lp = np.zeros(NSENT, dt)
    lp[0] = raw5[0, 0]
    ln = np.array([1, 0, 0], np.int32)
    tok = np.array([SOS, 0, 0], np.int32)
    alive = np.array([True, False, False])
    seqs = np.zeros((NSENT, seq_len_T), np.int32)
    seqs[0, 0] = SOS
    fsc = np.full(NSENT, BIGF, dt)
    fseq = np.zeros((NSENT, seq_len_T), np.int32)

    for t in range(1, seq_len_T):
        raw = raw5[t]
        toki = topi[t]
        is_eos = alive & (tok == EOS) & (ln > 1)
        if is_eos.any():
            ev = -(lp / (ln.astype(dt) - F1 + EPS))
            cand_f = np.where(is_eos, ev, BIGF)
            all_sc = np.concatenate([fsc, cand_f])
            all_seq = np.concatenate([fseq, seqs], axis=0)
            keep = np.argsort(all_sc, kind="stable")[:NSENT]
            fsc, fseq = all_sc[keep], all_seq[keep]
        expand = alive & ~is_eos
        cl = lp[:, None] + raw[None, :]
        clen = ln[:, None] + 1  # shape [3, 1], as in the reference
        csc = -(cl / (clen.astype(dt) - F1 + EPS))
        csc = np.where(expand[:, None], csc, BIGF).reshape(-1)
        sel = np.argsort(csc, kind="stable")[:NSENT]
        parent = sel // BEAM_WIDTH
        lp = cl.reshape(-1)[sel]
        # reference does clen.reshape(-1)[sel] on a 3-element array under
        # JAX out-of-bounds clamping semantics
        ln = clen.reshape(-1)[np.minimum(sel, NSENT - 1)]
        tok = toki[sel % BEAM_WIDTH]
        alive = csc[sel] < HALF_BIG
        seqs = seqs[parent].copy()
        seqs[:, t] = tok

    beam_sc = np.where(alive, -(lp / (ln.astype(dt) - F1 + EPS)), BIGF)
    rank_sc = np.concatenate([fsc, beam_sc + OFFS])
    true_sc = np.concatenate([fsc, beam_sc])
    all_seq = np.concatenate([fseq, seqs], axis=0)
    order = np.argsort(rank_sc, kind="stable")[:NSENT]
    return true_sc[order], all_seq[order]


def kernel(predicted_seq_lprobs, target_seq, seq_len):
    lp_full = np.ascontiguousarray(
        np.asarray(predicted_seq_lprobs, dtype=np.float32)[0]
    )  # [T, V]
    segv, segi, _ = _device_seg8(lp_full)
    topi = _host_top5_from_segments(lp_full, segv, segi)
    raw5 = lp_full[:, :BEAM_WIDTH]
    scores, seqs = _host_beam_scan(raw5, topi, T)
    return seqs.astype(np.int32), scores.astype(np.float32)


# revision 12
# speedup vs baseline: 1.2069x; 1.0372x over previous
"""Trainium2 Bass kernel for nn_BeamSearchDecoder_46093589021017.

Structure of the problem: the reference beam search over lprobs [1, T=4096,
V=10000] only ever consumes
  - topi[t] = indices of the top-5 lprobs of row t     (the heavy part), and
  - raw5[t] = lprobs[t, :5]                            (a pure slice),
after which a tiny sequential 3-beam scan (T-1 steps over [3,5] candidates)
produces the output. The memory-bound part (reading 164 MB once) runs on the
8 NeuronCores, sharded over the T axis (512 rows/core); the steady state sits
on the device HBM roofline (~2.9 TB/s aggregate).

Device algorithm per row (exact, one DVE pass over the data):
  1. segmented max-reduce: 400 segment maxes of width 25 (1 elem/cycle DVE)
  2. row-tiles 0..2: max8 + find_index8 over the 400 segment maxes -> top-8
     segment ids (the ids alone suffice: duplicated needle values latch the
     same first-match position, so ties are detectable from duplicate ids)
  3. row-tile 3 (the last): its segment maxes are streamed to DRAM per chunk
     as they are reduced, so the kernel tail has no finalize/handoff; the
     host finds that tile's candidate segments exactly.
Host: the true top-5 values of a row live in segments whose max is >= the
5th-largest segment max (s5 <= v5 always), i.e. within the top-8 segments.
Gather those 8x25 columns from the input (host already holds it) and take
the exact top-5. Rows whose candidate-segment set is ambiguous (duplicate
ids from value ties / >8 segments above threshold) fall back to an exact
host re-scan of the row; with continuous data this never triggers.

The [T,5] -> ([3],[3,T]) beam scan is inherently sequential tiny bookkeeping
and runs on host in exact float32, replicating the reference op-for-op
(including JAX's out-of-bounds index clamping).
"""

import numpy as np

import concourse.bass as bass
import concourse.mybir as mybir
from concourse.bass_utils import run_bass_kernel_spmd

# Problem shapes (hardcoded per contract; kernel.py must be self-contained).
T = 4096
V = 10000
N_CORES = 8
ROWS = T // N_CORES  # rows (timesteps) per core
P = 128  # SBUF partitions
N_TILES = ROWS // P  # row-tiles per core

SW = 25  # segment width for the first-level max-reduce
SEGS = V // SW  # 400 segments per row
W = 2500  # columns per load chunk
CHUNKS = V // W  # 4 chunks per row-tile
SPC = W // SW  # segments per chunk
NSLOTS = 8  # load buffering depth

BEAM_WIDTH = 5
NSENT = 3
SOS = 1
EOS = 2

_BASS_CACHE = {}


def _build_topk_bass():
    """One-core Bass program. Outputs:
      segi [(N_TILES-1)*P, 8] uint32 — top-8 segment ids for row-tiles 0..2
      seg3 [P, SEGS] f32           — full segment maxes for row-tile 3

    Hand-rolled pipeline (no TileContext: its final drain would need 9 sem
    waits — DVE + 8 DMA-HW queues — exceeding the codegen per-instruction
    wait limit). Loads on SP, reduce/max8/find-index8 on DVE, stores on
    Activation. One load semaphore per buffer slot: descriptor completions
    of different in-flight DMAs interleave on a shared semaphore, so a
    shared counter cannot tell which DMA finished.
    """
    nc = bass.Bass("TRN2")
    lp = nc.declare_dram_parameter("lp", [ROWS, V], mybir.dt.float32, isOutput=False)
    outi = nc.declare_dram_parameter(
        "segi", [(N_TILES - 1) * P, 8], mybir.dt.uint32, isOutput=True
    )
    out3 = nc.declare_dram_parameter("seg3", [P, SEGS], mybir.dt.float32, isOutput=True)
    bufs = [
        nc.alloc_sbuf_tensor(f"buf{s}", [P, W], mybir.dt.float32)
        for s in range(NSLOTS)
    ]
    segmax = [
        nc.alloc_sbuf_tensor(f"segmax{t}", [P, SEGS], mybir.dt.float32)
        for t in range(N_TILES)
    ]
    segv = [
        nc.alloc_sbuf_tensor(f"segv{t}", [P, 8], mybir.dt.float32)
        for t in range(N_TILES - 1)
    ]
    segi = [
        nc.alloc_sbuf_tensor(f"segi{t}", [P, 8], mybir.dt.uint32)
        for t in range(N_TILES - 1)
    ]
    NCHUNKS = N_TILES * CHUNKS
    with (
        nc.Block() as block,
        nc.semaphore("load_sem0") as load_sem0,
        nc.semaphore("load_sem1") as load_sem1,
        nc.semaphore("load_sem2") as load_sem2,
        nc.semaphore("load_sem3") as load_sem3,
        nc.semaphore("load_sem4") as load_sem4,
        nc.semaphore("load_sem5") as load_sem5,
        nc.semaphore("load_sem6") as load_sem6,
        nc.semaphore("load_sem7") as load_sem7,
        nc.semaphore("red_sem") as red_sem,
        nc.semaphore("max_sem") as max_sem,
        nc.semaphore("vec_sem") as vec_sem,
        nc.semaphore("store_sem") as store_sem,
    ):
        load_sems = [
            load_sem0,
            load_sem1,
            load_sem2,
            load_sem3,
            load_sem4,
            load_sem5,
            load_sem6,
            load_sem7,
        ]

        @block.sync
        def _(sync):
            for ci in range(NCHUNKS):
                if ci >= NSLOTS:
                    # slot ci%NSLOTS is free once DVE reduced chunk ci-NSLOTS
                    sync.wait_ge(red_sem, ci - NSLOTS + 1)
                t, c = divmod(ci, CHUNKS)
                sync.dma_start(
                    out=bufs[ci % NSLOTS][:],
                    in_=lp[t * P : (t + 1) * P, c * W : (c + 1) * W],
                ).then_inc(load_sems[ci % NSLOTS], 16)

        @block.vector
        def _(vector):
            for ci in range(NCHUNKS):
                t, c = divmod(ci, CHUNKS)
                s = ci % NSLOTS
                vector.wait_ge(load_sems[s], 16 * (ci // NSLOTS + 1))
                vector.reduce_max(
                    out=segmax[t][:, c * SPC : (c + 1) * SPC],
                    in_=bufs[s][:].rearrange("p (g w) -> p g w", w=SW),
                    axis=mybir.AxisListType.X,
                ).then_inc(red_sem, 1)
                if c == CHUNKS - 1 and t < N_TILES - 1:
                    vector.wait_ge(red_sem, ci + 1)
                    vector.max(out=segv[t][:], in_=segmax[t][:]).then_inc(max_sem, 1)
                    vector.wait_ge(max_sem, t + 1)
                    vector.max_index(
                        out=segi[t][:], in_max=segv[t][:], in_values=segmax[t][:]
                    ).then_inc(vec_sem, 1)

        @block.scalar
        def _(act):
            for t in range(N_TILES - 1):
                act.wait_ge(vec_sem, t + 1)
                act.dma_start(
                    out=outi[t * P : (t + 1) * P, :], in_=segi[t][:]
                ).then_inc(store_sem, 16)
            t3 = N_TILES - 1
            for c in range(CHUNKS):
                act.wait_ge(red_sem, t3 * CHUNKS + c + 1)
                act.dma_start(
                    out=out3[:, c * SPC : (c + 1) * SPC],
                    in_=segmax[t3][:, c * SPC : (c + 1) * SPC],
                ).then_inc(store_sem, 16)
            act.wait_ge(store_sem, 16 * (N_TILES - 1 + CHUNKS))

    return nc


def _device_seg(lp_full: np.ndarray, trace: bool = False):
    """lp_full: [T, V] f32 -> (segi [(NT-1)*P*8cores, 8] u32,
    seg3 [P*8cores, SEGS] f32, exec_ns). Row mapping: core c contributes
    global rows c*ROWS..c*ROWS+384 (segi) and c*ROWS+384..(c+1)*ROWS (seg3)."""
    if "nc" not in _BASS_CACHE:
        _BASS_CACHE["nc"] = _build_topk_bass()
    nc = _BASS_CACHE["nc"]
    in_maps = [
        {"lp": np.ascontiguousarray(lp_full[c * ROWS : (c + 1) * ROWS])}
        for c in range(N_CORES)
    ]
    out = run_bass_kernel_spmd(nc, in_maps, core_ids=list(range(N_CORES)), trace=trace)
    segi = np.concatenate([out.results[c]["segi"] for c in range(N_CORES)], axis=0)
    seg3 = np.concatenate([out.results[c]["seg3"] for c in range(N_CORES)], axis=0)
    return segi, seg3, out.exec_time_ns


def _host_top5(lp_full, segi_all, seg3_all):
    """Exact per-row top-5 indices from the device outputs."""
    n = lp_full.shape[0]
    DEVP = (N_TILES - 1) * P  # device-finalized rows per core
    cand = np.empty((n, 8), np.int64)
    bad = np.zeros(n, bool)

    idx_dev = np.concatenate(
        [np.arange(c * ROWS, c * ROWS + DEVP) for c in range(N_CORES)]
    )
    si = segi_all.astype(np.int64)
    ssort = np.sort(si, axis=1)
    # duplicate ids <=> tied top-8 segment-max values (find_index8 latches the
    # same first match for equal needles); either voids the coverage proof
    bad_dev = (ssort[:, 1:] == ssort[:, :-1]).any(axis=1) | (si >= SEGS).any(axis=1)
    cand[idx_dev] = si
    bad[idx_dev] = bad_dev

    idx_h = np.concatenate(
        [np.arange(c * ROWS + DEVP, (c + 1) * ROWS) for c in range(N_CORES)]
    )
    sm3 = seg3_all  # [1024, SEGS] f32, exact segment maxes
    s5 = np.partition(sm3, SEGS - 5, axis=1)[:, SEGS - 5]
    cnt = (sm3 >= s5[:, None]).sum(axis=1)
    cand[idx_h] = np.argpartition(-sm3, 8, axis=1)[:, :8]
    bad[idx_h] = cnt > 8  # >8 segments above the s5 threshold: tie pile-up

    cand.sort(axis=1)  # ascending column order => stable ties pick lowest idx
    cols = (
        cand[:, :, None] * SW + np.arange(SW, dtype=np.int64)[None, None, :]
    ).reshape(n, 8 * SW)
    vals = np.take_along_axis(lp_full, cols, axis=1)
    ord5 = np.argsort(-vals, axis=1, kind="stable")[:, :BEAM_WIDTH]
    topi = np.take_along_axis(cols, ord5, axis=1).astype(np.int32)
    for r in np.flatnonzero(bad):
        topi[r] = np.argsort(-lp_full[r], kind="stable")[:BEAM_WIDTH]
    return topi


def _host_beam_scan(raw5: np.ndarray, topi: np.ndarray, seq_len_T: int):
    """Exact float32 port of the reference _beam_search scan, consuming
    precomputed raw5 [T,5] (lprobs[:, :5]) and topi [T,5] (top-5 indices)."""
    dt = np.float32
    BIGF = np.float32(1e30)
    F1 = np.float32(1.0)
    EPS = np.float32(1e-6)
    HALF_BIG = np.float32(1e30 * 0.5)
    OFFS = np.float32(1e15)

    lp = np.zeros(NSENT, dt)
    lp[0] = raw5[0, 0]
    ln = np.array([1, 0, 0], np.int32)
    tok = np.array([SOS, 0, 0], np.int32)
    alive = np.array([True, False, False])
    seqs = np.zeros((NSENT, seq_len_T), np.int32)
    seqs[0, 0] = SOS
    fsc = np.full(NSENT, BIGF, dt)
    fseq = np.zeros((NSENT, seq_len_T), np.int32)

    for t in range(1, seq_len_T):
        raw = raw5[t]
        toki = topi[t]
        is_eos = alive & (tok == EOS) & (ln > 1)
        if is_eos.any():
            ev = -(lp / (ln.astype(dt) - F1 + EPS))
            cand_f = np.where(is_eos, ev, BIGF)
            all_sc = np.concatenate([fsc, cand_f])
            all_seq = np.concatenate([fseq, seqs], axis=0)
            keep = np.argsort(all_sc, kind="stable")[:NSENT]
            fsc, fseq = all_sc[keep], all_seq[keep]
        expand = alive & ~is_eos
        cl = lp[:, None] + raw[None, :]
        clen = ln[:, None] + 1  # shape [3, 1], as in the reference
        csc = -(cl / (clen.astype(dt) - F1 + EPS))
        csc = np.where(expand[:, None], csc, BIGF).reshape(-1)
        sel = np.argsort(csc, kind="stable")[:NSENT]
        parent = sel // BEAM_WIDTH
        lp = cl.reshape(-1)[sel]
        # reference does clen.reshape(-1)[sel] on a 3-element array under
        # JAX out-of-bounds clamping semantics
        ln = clen.reshape(-1)[np.minimum(sel, NSENT - 1)]
        tok = toki[sel % BEAM_WIDTH]
        alive = csc[sel] < HALF_BIG
        seqs = seqs[parent].copy()
        seqs[:, t] = tok

    beam_sc = np.where(alive, -(lp / (ln.astype(dt) - F1 + EPS)), BIGF)
    rank_sc = np.concatenate([fsc, beam_sc + OFFS])
    true_sc = np.concatenate([fsc, beam_sc])
    all_seq = np.concatenate([fseq, seqs], axis=0)
    order = np.argsort(rank_sc, kind="stable")[:NSENT]
    return true_sc[order], all_seq[order]


def kernel(predicted_seq_lprobs, target_seq, seq_len):
    lp_full = np.ascontiguousarray(
        np.asarray(predicted_seq_lprobs, dtype=np.float32)[0]
    )  # [T, V]
    segi, seg3, _ = _device_seg(lp_full)
    topi = _host_top5(lp_full, segi, seg3)
    raw5 = lp_full[:, :BEAM_WIDTH]
    scores, seqs = _host_beam_scan(raw5, topi, T)
    return seqs.astype(np.int32), scores.astype(np.float32)
